# revision 10
# baseline (speedup 1.0000x reference)
"""Trainium2 Bass kernel for Conv2D(sum of 20 1x1 convs) + QwenRMSNorm.

Math: y = einsum("bsi,loi->bso", x, conv_w) / L ; out = rmsnorm(y) * norm_w.
Since x does not depend on l, the 20-matrix contraction collapses to a single
matmul with W = sum_l conv_w[l] / L.  Host pre-sums/transposes/casts the weight
(one [H,H] matrix) and lays out x as token-sharded, hidden-major bf16 slabs;
the 8 NeuronCores each run matmul (bf16, fp32 accum) + RMSNorm on their 2048
tokens.  All device compute is token-local; no collectives.

v2 layout/schedule notes:
 - DRAM layouts are partition-outermost so multi-tile DMA chunks are
   contiguous per partition (x in 4 chunks, w in 4 chunks split across both
   HWDGE rings so the critical w+x0 fill gets full HBM bandwidth).
 - ~10 warm-up matmuls on a memset tile run during the DMA fill so the PE
   HAM clock-gate is at 8/8 before the first real matmul.
 - A dummy activation preloads the ACT function table during the fill.
 - Output is written bf16 (host upcasts); rel-err cost ~1e-3.
 - norm_w == 1 (the spec's fill) skips the [128,H] norm_w broadcast and the
   per-tile tensor_tensor multiply; a general variant handles arbitrary
   norm_w.
"""

import numpy as np
import ml_dtypes
from contextlib import ExitStack

import concourse.bass as bass
import concourse.mybir as mybir
import concourse.tile as tile
from concourse.bass_utils import run_bass_kernel_spmd

N_CORES = 8
B, S, H, L = 4, 4096, 1024, 20
TOK = B * S               # 16384 tokens
TPC = TOK // N_CORES      # 2048 tokens per core
TB = TPC // 128           # 16 token-blocks of 128 per core
KB = H // 128             # 8 contraction blocks
NOH = H // 512            # 2 psum halves of the output row
EPS = 1e-6
N_WARM = 5                # HAM warm-up matmuls (N=512) during the DMA fill

BF16 = mybir.dt.bfloat16
F32 = mybir.dt.float32
AF = mybir.ActivationFunctionType
OP = mybir.AluOpType

_BUILT = {}          # variant -> cached Bass program
LAST_RESULTS = None  # BassKernelResults of the most recent run (for test harness)


def _legalize_multiwait(nc):
    """The walrus build here encodes exactly one semaphore wait per 64B
    instruction (NEURON_ISA_TPB_EVENTS has a single wait slot) and errors on
    Tile's multi-wait instructions.  Split surplus waits into standalone
    EVENT_SEMAPHORE instructions on the same engine, placed directly before
    the original instruction (same sequencer stream -> same semantics)."""
    n_ev = 0
    for f in nc.m.functions:
        for blk in f.blocks:
            insts = blk.instructions
            out = []
            changed = False
            for inst in list(insts):
                si = getattr(inst, "sync_info", None)
                waits = list(si.on_wait) if si is not None else []
                if len(waits) > 1:
                    changed = True
                    updates = list(si.on_update)
                    for w in waits[:-1]:
                        ev = mybir.InstEventSemaphore(
                            name=f"{inst.name}-sw{n_ev}", ins=[], outs=[])
                        n_ev += 1
                        ev.engine = inst.engine
                        ev.sync_info = mybir.SyncInfo(on_wait=[w], on_update=[])
                        out.append(ev)
                    inst.sync_info = mybir.SyncInfo(
                        on_wait=[waits[-1]], on_update=updates)
                out.append(inst)
            if changed:
                insts.clear()
                insts.extend(out)


def _build(with_nw):
    nc = bass.Bass()
    # x layout (partition-outermost): xt[p, tt, ib, t] = x[tt*128+t, ib*128+p]
    xt_h = nc.dram_tensor("xt", [128, TB, KB, 128], BF16, kind="ExternalInput")
    # w layout: wt[p, ib, oh, j] = W[oh*512+j, ib*128+p], W = sum_l conv_w[l]/L
    # (ib-major so one 512KB chunk = an ib-pair for BOTH output halves, with
    # 4KB contiguous per partition)
    wt_h = nc.dram_tensor("wt", [128, KB, NOH, 512], BF16, kind="ExternalInput")
    if with_nw:
        nw_h = nc.dram_tensor("nw", [H], F32, kind="ExternalInput")
    out_h = nc.dram_tensor("out", [TPC, H], BF16, kind="ExternalOutput")

    with tile.TileContext(nc) as tc, ExitStack() as ctx:
        xpool = ctx.enter_context(tc.tile_pool(name="x", bufs=1))
        wpool = ctx.enter_context(tc.tile_pool(name="w", bufs=1))
        cpool = ctx.enter_context(tc.tile_pool(name="consts", bufs=1))
        opool = ctx.enter_context(tc.tile_pool(name="out", bufs=4))
        spool = ctx.enter_context(tc.tile_pool(name="scratch", bufs=2))
        stats = ctx.enter_context(tc.tile_pool(name="stats", bufs=8))
        psum = ctx.enter_context(tc.tile_pool(name="psum", bufs=4, space="PSUM"))

        # const tiles first so the warm-up matmuls' source is ready early
        wu_sb = cpool.tile([128, 512], BF16)
        nc.vector.memset(wu_sb, 0.0)
        zero_sb = cpool.tile([128, 1], F32)
        nc.vector.memset(zero_sb, 0.0)
        eps_sb = cpool.tile([128, 1], F32)
        nc.vector.memset(eps_sb, EPS)

        x_sb = xpool.tile([128, TB, KB, 128], BF16)
        w_sb = wpool.tile([128, KB, NOH, 512], BF16)

        # DMA schedule: the critical fill (x tile0 + all of w) alternates
        # w ib-pair chunks across both HWDGE rings in consumption order;
        # bulk x rides the third (SWDGE/gpsimd) queue so it doesn't steal
        # ring time from the critical chunks.
        nc.sync.dma_start(out=x_sb[:, 0:1], in_=xt_h[:, 0:1])
        nc.scalar.dma_start(out=w_sb[:, 0:2], in_=wt_h[:, 0:2])
        nc.sync.dma_start(out=w_sb[:, 2:4], in_=wt_h[:, 2:4])
        nc.scalar.dma_start(out=w_sb[:, 4:6], in_=wt_h[:, 4:6])
        nc.sync.dma_start(out=w_sb[:, 6:8], in_=wt_h[:, 6:8])
        nc.scalar.dma_start(out=x_sb[:, 1:2], in_=xt_h[:, 1:2])
        nc.scalar.dma_start(out=x_sb[:, 2:4], in_=xt_h[:, 2:4])
        nc.gpsimd.dma_start(out=x_sb[:, 4:8], in_=xt_h[:, 4:8])
        nc.gpsimd.dma_start(out=x_sb[:, 8:16], in_=xt_h[:, 8:16])

        if with_nw:
            # norm_w broadcast to 128 partitions (general path only)
            nw_sb = cpool.tile([128, H], F32)
            nc.gpsimd.dma_start(
                out=nw_sb,
                in_=bass.AP(tensor=nw_h, offset=0, ap=[[0, 128], [1, H]]))

        # preload the ACT function table (Square/Sqrt) during the fill
        dummy = stats.tile([128, 1], F32)
        nc.scalar.activation(out=dummy, in_=zero_sb, func=AF.Square,
                             bias=zero_sb)

        # HAM warm-up: keep the PE busy from ~7us until the real matmuls
        # start so the clock gate opens to 8/8 before real work.
        wp = psum.tile([128, H], F32, name="wp", tag="yp")
        for _ in range(N_WARM):
            nc.tensor.matmul(wp[:, 0:512], wu_sb[:, 0:128], wu_sb,
                             start=True, stop=True)

        for tt in range(TB):
            yp = psum.tile([128, H], F32, tag="yp")
            for oh in range(NOH):
                for ib in range(KB):
                    nc.tensor.matmul(
                        yp[:, oh * 512:(oh + 1) * 512],
                        x_sb[:, tt, ib, :],
                        w_sb[:, ib, oh, :],
                        start=(ib == 0),
                        stop=(ib == KB - 1),
                    )
            # sum of squares over the hidden axis (free axis) on ACT,
            # one op per psum half so each waits on a single PE group
            sq = spool.tile([128, H], BF16)
            half_sums = stats.tile([128, 2], F32)
            for oh in range(NOH):
                sl = slice(oh * 512, (oh + 1) * 512)
                nc.scalar.activation(out=sq[:, sl], in_=yp[:, sl],
                                     func=AF.Square, bias=zero_sb,
                                     accum_out=half_sums[:, oh:oh + 1])
            ssum = stats.tile([128, 1], F32)
            nc.vector.tensor_add(out=ssum, in0=half_sums[:, 0:1],
                                 in1=half_sums[:, 1:2])
            # std = sqrt(mean + eps); rstd = 1/std
            std = stats.tile([128, 1], F32)
            nc.scalar.activation(out=std, in_=ssum, func=AF.Sqrt,
                                 bias=eps_sb, scale=1.0 / H)
            rstd = stats.tile([128, 1], F32)
            nc.vector.reciprocal(out=rstd, in_=std)
            # out = (y * rstd) [* norm_w], written bf16; the two halves run
            # on different engines and leave via different DMA rings so the
            # last tile's drain is short
            o_sb = opool.tile([128, H], BF16)
            if with_nw:
                for oh in range(NOH):
                    sl = slice(oh * 512, (oh + 1) * 512)
                    nc.vector.scalar_tensor_tensor(
                        out=o_sb[:, sl], in0=yp[:, sl], scalar=rstd,
                        in1=nw_sb[:, sl], op0=OP.mult, op1=OP.mult,
                    )
            else:
                nc.scalar.activation(out=o_sb[:, 0:512], in_=yp[:, 0:512],
                                     func=AF.Copy, scale=rstd)
                nc.vector.tensor_scalar_mul(out=o_sb[:, 512:1024],
                                            in0=yp[:, 512:1024],
                                            scalar1=rstd)
            rows = slice(tt * 128, (tt + 1) * 128)
            nc.sync.dma_start(out=out_h[rows, 0:512], in_=o_sb[:, 0:512])
            nc.scalar.dma_start(out=out_h[rows, 512:1024],
                                in_=o_sb[:, 512:1024])

    _legalize_multiwait(nc)
    return nc


def host_prep(x, conv_w, norm_w, with_nw):
    """Shard + lay out the full inputs into per-core device input maps."""
    bf16 = ml_dtypes.bfloat16

    # Collapse the 20 1x1 convs: W[o,i] = sum_l conv_w[l,o,i] / L
    w = np.asarray(conv_w).sum(axis=0) * (1.0 / L)          # [H(o), H(i)] f32
    # wt[p, ib, oh, j] = W[oh*512+j, ib*128+p]
    wt = np.ascontiguousarray(
        w.reshape(NOH, 512, KB, 128).transpose(3, 2, 0, 1).astype(bf16))

    x2d = np.asarray(x).reshape(TOK, H)
    xbf = x2d.astype(bf16)

    in_maps = []
    for c in range(N_CORES):
        xc = xbf[c * TPC:(c + 1) * TPC]                      # [TPC, H]
        # xt[p, tt, ib, t] = xc[tt*128+t, ib*128+p]
        xtc = np.ascontiguousarray(
            xc.reshape(TB, 128, KB, 128).transpose(3, 0, 2, 1))
        m = {"xt": xtc, "wt": wt}
        if with_nw:
            m["nw"] = np.ascontiguousarray(np.asarray(norm_w),
                                           dtype=np.float32)
        in_maps.append(m)
    return in_maps


def kernel(x, conv_w, norm_w):
    global LAST_RESULTS

    x = np.asarray(x)
    out_dtype = x.dtype
    nw = np.asarray(norm_w)
    with_nw = not bool(np.all(nw == 1.0))

    if with_nw not in _BUILT:
        _BUILT[with_nw] = _build(with_nw)
    nc = _BUILT[with_nw]

    in_maps = host_prep(x, conv_w, norm_w, with_nw)

    res = run_bass_kernel_spmd(nc, in_maps, core_ids=list(range(N_CORES)))
    LAST_RESULTS = res

    out = np.concatenate([r["out"] for r in res.results], axis=0)
    return out.reshape(B, S, H).astype(out_dtype, copy=False)


# revision 16
# speedup vs baseline: 1.1227x; 1.1227x over previous
"""Trainium2 Bass kernel for Conv2D(sum of 20 1x1 convs) + QwenRMSNorm.

Math: y = einsum("bsi,loi->bso", x, conv_w) / L ; out = rmsnorm(y) * norm_w.
Since x does not depend on l, the 20-matrix contraction collapses to a single
matmul with W = sum_l conv_w[l] / L.  Host pre-sums/transposes/casts the weight
(one [H,H] matrix) and lays out x as token-sharded, hidden-major bf16 slabs;
the 8 NeuronCores each run matmul (bf16, fp32 accum) + RMSNorm on their 2048
tokens.  All device compute is token-local; no collectives.

v2 layout/schedule notes:
 - DRAM layouts are partition-outermost so multi-tile DMA chunks are
   contiguous per partition (x in 4 chunks, w in 4 chunks split across both
   HWDGE rings so the critical w+x0 fill gets full HBM bandwidth).
 - ~10 warm-up matmuls on a memset tile run during the DMA fill so the PE
   HAM clock-gate is at 8/8 before the first real matmul.
 - A dummy activation preloads the ACT function table during the fill.
 - Output is written bf16 (host upcasts); rel-err cost ~1e-3.
 - norm_w == 1 (the spec's fill) skips the [128,H] norm_w broadcast and the
   per-tile tensor_tensor multiply; a general variant handles arbitrary
   norm_w.
"""

import numpy as np
import ml_dtypes
from contextlib import ExitStack

import concourse.bass as bass
import concourse.mybir as mybir
import concourse.tile as tile
from concourse.bass_utils import run_bass_kernel_spmd

N_CORES = 8
B, S, H, L = 4, 4096, 1024, 20
TOK = B * S               # 16384 tokens
TPC = TOK // N_CORES      # 2048 tokens per core
TB = TPC // 128           # 16 token-blocks of 128 per core
KB = H // 128             # 8 contraction blocks
NOH = H // 512            # 2 psum halves of the output row
EPS = 1e-6
N_WARM = 5                # HAM warm-up matmuls (N=512) during the DMA fill

BF16 = mybir.dt.bfloat16
F32 = mybir.dt.float32
AF = mybir.ActivationFunctionType
OP = mybir.AluOpType

_BUILT = {}          # variant -> cached Bass program
LAST_RESULTS = None  # BassKernelResults of the most recent run (for test harness)


def _legalize_multiwait(nc):
    """The walrus build here encodes exactly one semaphore wait per 64B
    instruction (NEURON_ISA_TPB_EVENTS has a single wait slot) and errors on
    Tile's multi-wait instructions.  Split surplus waits into standalone
    EVENT_SEMAPHORE instructions on the same engine, placed directly before
    the original instruction (same sequencer stream -> same semantics)."""
    n_ev = 0
    for f in nc.m.functions:
        for blk in f.blocks:
            insts = blk.instructions
            out = []
            changed = False
            for inst in list(insts):
                si = getattr(inst, "sync_info", None)
                waits = list(si.on_wait) if si is not None else []
                if len(waits) > 1:
                    changed = True
                    updates = list(si.on_update)
                    for w in waits[:-1]:
                        ev = mybir.InstEventSemaphore(
                            name=f"{inst.name}-sw{n_ev}", ins=[], outs=[])
                        n_ev += 1
                        ev.engine = inst.engine
                        ev.sync_info = mybir.SyncInfo(on_wait=[w], on_update=[])
                        out.append(ev)
                    inst.sync_info = mybir.SyncInfo(
                        on_wait=[waits[-1]], on_update=updates)
                out.append(inst)
            if changed:
                insts.clear()
                insts.extend(out)


def _build(with_nw):
    nc = bass.Bass()
    # x layout (partition-outermost): xt[p, tt, ib, t] = x[tt*128+t, ib*128+p]
    xt_h = nc.dram_tensor("xt", [128, TB, KB, 128], BF16, kind="ExternalInput")
    # w layout: wt[p, ib, oh, j] = W[oh*512+j, ib*128+p], W = sum_l conv_w[l]/L
    # (ib-major so one 512KB chunk = an ib-pair for BOTH output halves, with
    # 4KB contiguous per partition)
    wt_h = nc.dram_tensor("wt", [128, KB, NOH, 512], BF16, kind="ExternalInput")
    if with_nw:
        nw_h = nc.dram_tensor("nw", [H], F32, kind="ExternalInput")
    out_h = nc.dram_tensor("out", [TPC, H], BF16, kind="ExternalOutput")

    with tile.TileContext(nc) as tc, ExitStack() as ctx:
        xpool = ctx.enter_context(tc.tile_pool(name="x", bufs=1))
        wpool = ctx.enter_context(tc.tile_pool(name="w", bufs=1))
        cpool = ctx.enter_context(tc.tile_pool(name="consts", bufs=1))
        opool = ctx.enter_context(tc.tile_pool(name="out", bufs=6))
        spool = ctx.enter_context(tc.tile_pool(name="scratch", bufs=2))
        stats = ctx.enter_context(tc.tile_pool(name="stats", bufs=8))
        psum = ctx.enter_context(tc.tile_pool(name="psum", bufs=4, space="PSUM"))

        # const tiles first so the warm-up matmuls' source is ready early
        wu_sb = cpool.tile([128, 512], BF16)
        nc.vector.memset(wu_sb, 0.0)
        zero_sb = cpool.tile([128, 1], F32)
        nc.vector.memset(zero_sb, 0.0)
        eps_sb = cpool.tile([128, 1], F32)
        nc.vector.memset(eps_sb, EPS)

        x_sb = xpool.tile([128, TB, KB, 128], BF16)
        w_sb = wpool.tile([128, KB, NOH, 512], BF16)

        # DMA schedule: the critical fill (x tile0 + all of w) alternates
        # w ib-pair chunks across both HWDGE rings in consumption order.
        # Bulk x for tiles 8-15 rides the third (SWDGE/gpsimd) queue, but
        # only after w has landed — the tiny gpsimd copy below creates that
        # cross-queue ordering so SWDGE traffic cannot starve the critical
        # fill (it steals most of the HBM bandwidth if launched at t=0).
        nc.sync.dma_start(out=x_sb[:, 0:1], in_=xt_h[:, 0:1])
        nc.scalar.dma_start(out=w_sb[:, 0:2], in_=wt_h[:, 0:2])
        nc.sync.dma_start(out=w_sb[:, 2:4], in_=wt_h[:, 2:4])
        nc.scalar.dma_start(out=w_sb[:, 4:6], in_=wt_h[:, 4:6])
        nc.sync.dma_start(out=w_sb[:, 6:8], in_=wt_h[:, 6:8])
        nc.scalar.dma_start(out=x_sb[:, 1:2], in_=xt_h[:, 1:2])
        nc.scalar.dma_start(out=x_sb[:, 2:4], in_=xt_h[:, 2:4])
        nc.sync.dma_start(out=x_sb[:, 4:8], in_=xt_h[:, 4:8])
        wgate = cpool.tile([128, 8], BF16)
        nc.gpsimd.tensor_copy(out=wgate, in_=w_sb[:, 0:8, 0, 0:1])
        nc.gpsimd.dma_start(out=x_sb[:, 8:10], in_=xt_h[:, 8:10])
        nc.gpsimd.dma_start(out=x_sb[:, 10:12], in_=xt_h[:, 10:12])
        nc.gpsimd.dma_start(out=x_sb[:, 12:14], in_=xt_h[:, 12:14])
        nc.gpsimd.dma_start(out=x_sb[:, 14:16], in_=xt_h[:, 14:16])

        if with_nw:
            # norm_w broadcast to 128 partitions (general path only)
            nw_sb = cpool.tile([128, H], F32)
            nc.gpsimd.dma_start(
                out=nw_sb,
                in_=bass.AP(tensor=nw_h, offset=0, ap=[[0, 128], [1, H]]))

        # preload the ACT function table (Square/Sqrt) during the fill
        dummy = stats.tile([128, 1], F32)
        nc.scalar.activation(out=dummy, in_=zero_sb, func=AF.Square,
                             bias=zero_sb)

        # HAM warm-up: keep the PE busy from ~7us until the real matmuls
        # start so the clock gate opens to 8/8 before real work.
        wp = psum.tile([128, H], F32, name="wp", tag="yp")
        for _ in range(N_WARM):
            nc.tensor.matmul(wp[:, 0:512], wu_sb[:, 0:128], wu_sb,
                             start=True, stop=True)

        for tt in range(TB):
            yp = psum.tile([128, H], F32, tag="yp")
            for oh in range(NOH):
                for ib in range(KB):
                    nc.tensor.matmul(
                        yp[:, oh * 512:(oh + 1) * 512],
                        x_sb[:, tt, ib, :],
                        w_sb[:, ib, oh, :],
                        start=(ib == 0),
                        stop=(ib == KB - 1),
                    )
            # sum of squares over the hidden axis (free axis) on ACT,
            # one op per psum half so each waits on a single PE group
            sq = spool.tile([128, H], BF16, tag="sq")
            half_sums = stats.tile([128, 2], F32)
            for oh in range(NOH):
                sl = slice(oh * 512, (oh + 1) * 512)
                nc.scalar.activation(out=sq[:, sl], in_=yp[:, sl],
                                     func=AF.Square, bias=zero_sb,
                                     accum_out=half_sums[:, oh:oh + 1])
            ssum = stats.tile([128, 1], F32)
            nc.vector.tensor_add(out=ssum, in0=half_sums[:, 0:1],
                                 in1=half_sums[:, 1:2])
            # std = sqrt(mean + eps); rstd = 1/std
            std = stats.tile([128, 1], F32)
            nc.scalar.activation(out=std, in_=ssum, func=AF.Sqrt,
                                 bias=eps_sb, scale=1.0 / H)
            rstd = stats.tile([128, 1], F32)
            nc.vector.reciprocal(out=rstd, in_=std)
            # out = (y * rstd) [* norm_w], written bf16 on DVE; the two
            # halves leave via different DMA rings so the last tile drains
            # fast
            o_sb = opool.tile([128, H], BF16)
            if with_nw:
                for oh in range(NOH):
                    sl = slice(oh * 512, (oh + 1) * 512)
                    nc.vector.scalar_tensor_tensor(
                        out=o_sb[:, sl], in0=yp[:, sl], scalar=rstd,
                        in1=nw_sb[:, sl], op0=OP.mult, op1=OP.mult,
                    )
            else:
                for oh in range(NOH):
                    sl = slice(oh * 512, (oh + 1) * 512)
                    nc.vector.tensor_scalar_mul(out=o_sb[:, sl],
                                                in0=yp[:, sl],
                                                scalar1=rstd)
            rows = slice(tt * 128, (tt + 1) * 128)
            nc.sync.dma_start(out=out_h[rows, 0:512], in_=o_sb[:, 0:512])
            nc.scalar.dma_start(out=out_h[rows, 512:1024],
                                in_=o_sb[:, 512:1024])

    _legalize_multiwait(nc)
    return nc


def host_prep(x, conv_w, norm_w, with_nw):
    """Shard + lay out the full inputs into per-core device input maps."""
    bf16 = ml_dtypes.bfloat16

    # Collapse the 20 1x1 convs: W[o,i] = sum_l conv_w[l,o,i] / L
    w = np.asarray(conv_w).sum(axis=0) * (1.0 / L)          # [H(o), H(i)] f32
    # wt[p, ib, oh, j] = W[oh*512+j, ib*128+p]
    wt = np.ascontiguousarray(
        w.reshape(NOH, 512, KB, 128).transpose(3, 2, 0, 1).astype(bf16))

    x2d = np.asarray(x).reshape(TOK, H)
    xbf = x2d.astype(bf16)

    in_maps = []
    for c in range(N_CORES):
        xc = xbf[c * TPC:(c + 1) * TPC]                      # [TPC, H]
        # xt[p, tt, ib, t] = xc[tt*128+t, ib*128+p]
        xtc = np.ascontiguousarray(
            xc.reshape(TB, 128, KB, 128).transpose(3, 0, 2, 1))
        m = {"xt": xtc, "wt": wt}
        if with_nw:
            m["nw"] = np.ascontiguousarray(np.asarray(norm_w),
                                           dtype=np.float32)
        in_maps.append(m)
    return in_maps


def kernel(x, conv_w, norm_w):
    global LAST_RESULTS

    x = np.asarray(x)
    out_dtype = x.dtype
    nw = np.asarray(norm_w)
    with_nw = not bool(np.all(nw == 1.0))

    if with_nw not in _BUILT:
        _BUILT[with_nw] = _build(with_nw)
    nc = _BUILT[with_nw]

    in_maps = host_prep(x, conv_w, norm_w, with_nw)

    res = run_bass_kernel_spmd(nc, in_maps, core_ids=list(range(N_CORES)))
    LAST_RESULTS = res

    out = np.concatenate([r["out"] for r in res.results], axis=0)
    return out.reshape(B, S, H).astype(out_dtype, copy=False)


# revision 20
# speedup vs baseline: 1.1429x; 1.0181x over previous
"""Trainium2 Bass kernel for Conv2D(sum of 20 1x1 convs) + QwenRMSNorm.

Math: y = einsum("bsi,loi->bso", x, conv_w) / L ; out = rmsnorm(y) * norm_w.
Since x does not depend on l, the 20-matrix contraction collapses to a single
matmul with W = sum_l conv_w[l] / L.  Host pre-sums/transposes/casts the weight
(one [H,H] matrix) and lays out x as token-sharded, hidden-major bf16 slabs;
the 8 NeuronCores each run matmul (bf16, fp32 accum) + RMSNorm on their 2048
tokens.  All device compute is token-local; no collectives.

v2 layout/schedule notes:
 - DRAM layouts are partition-outermost so multi-tile DMA chunks are
   contiguous per partition (x in 4 chunks, w in 4 chunks split across both
   HWDGE rings so the critical w+x0 fill gets full HBM bandwidth).
 - ~10 warm-up matmuls on a memset tile run during the DMA fill so the PE
   HAM clock-gate is at 8/8 before the first real matmul.
 - A dummy activation preloads the ACT function table during the fill.
 - Output is written bf16 (host upcasts); rel-err cost ~1e-3.
 - norm_w == 1 (the spec's fill) skips the [128,H] norm_w broadcast and the
   per-tile tensor_tensor multiply; a general variant handles arbitrary
   norm_w.
"""

import numpy as np
import ml_dtypes
from contextlib import ExitStack

import concourse.bass as bass
import concourse.mybir as mybir
import concourse.tile as tile
from concourse.bass_utils import run_bass_kernel_spmd

N_CORES = 8
B, S, H, L = 4, 4096, 1024, 20
TOK = B * S               # 16384 tokens
TPC = TOK // N_CORES      # 2048 tokens per core
TB = TPC // 128           # 16 token-blocks of 128 per core
KB = H // 128             # 8 contraction blocks
NOH = H // 512            # 2 psum halves of the output row
EPS = 1e-6
N_WARM = 5                # HAM warm-up matmuls (N=512) during the DMA fill

BF16 = mybir.dt.bfloat16
F32 = mybir.dt.float32
AF = mybir.ActivationFunctionType
OP = mybir.AluOpType

_BUILT = {}          # variant -> cached Bass program
LAST_RESULTS = None  # BassKernelResults of the most recent run (for test harness)


def _legalize_multiwait(nc):
    """The walrus build here encodes exactly one semaphore wait per 64B
    instruction (NEURON_ISA_TPB_EVENTS has a single wait slot) and errors on
    Tile's multi-wait instructions.  Split surplus waits into standalone
    EVENT_SEMAPHORE instructions on the same engine, placed directly before
    the original instruction (same sequencer stream -> same semantics)."""
    n_ev = 0
    for f in nc.m.functions:
        for blk in f.blocks:
            insts = blk.instructions
            out = []
            changed = False
            for inst in list(insts):
                si = getattr(inst, "sync_info", None)
                waits = list(si.on_wait) if si is not None else []
                if len(waits) > 1:
                    changed = True
                    updates = list(si.on_update)
                    for w in waits[:-1]:
                        ev = mybir.InstEventSemaphore(
                            name=f"{inst.name}-sw{n_ev}", ins=[], outs=[])
                        n_ev += 1
                        ev.engine = inst.engine
                        ev.sync_info = mybir.SyncInfo(on_wait=[w], on_update=[])
                        out.append(ev)
                    inst.sync_info = mybir.SyncInfo(
                        on_wait=[waits[-1]], on_update=updates)
                out.append(inst)
            if changed:
                insts.clear()
                insts.extend(out)


def _build(with_nw):
    nc = bass.Bass()
    # x layout (partition-outermost): xt[p, tt, ib, t] = x[tt*128+t, ib*128+p]
    xt_h = nc.dram_tensor("xt", [128, TB, KB, 128], BF16, kind="ExternalInput")
    # w layout: wt[p, ib, oh, j] = W[oh*512+j, ib*128+p], W = sum_l conv_w[l]/L
    # (ib-major so one 512KB chunk = an ib-pair for BOTH output halves, with
    # 4KB contiguous per partition)
    wt_h = nc.dram_tensor("wt", [128, KB, NOH, 512], BF16, kind="ExternalInput")
    if with_nw:
        nw_h = nc.dram_tensor("nw", [H], F32, kind="ExternalInput")
    out_h = nc.dram_tensor("out", [TPC, H], BF16, kind="ExternalOutput")

    with tile.TileContext(nc) as tc, ExitStack() as ctx:
        xpool = ctx.enter_context(tc.tile_pool(name="x", bufs=1))
        wpool = ctx.enter_context(tc.tile_pool(name="w", bufs=1))
        cpool = ctx.enter_context(tc.tile_pool(name="consts", bufs=1))
        opool = ctx.enter_context(tc.tile_pool(name="out", bufs=6))
        spool = ctx.enter_context(tc.tile_pool(name="scratch", bufs=2))
        stats = ctx.enter_context(tc.tile_pool(name="stats", bufs=8))
        psum = ctx.enter_context(tc.tile_pool(name="psum", bufs=8, space="PSUM"))

        # const tiles first so the warm-up matmuls' source is ready early
        wu_sb = cpool.tile([128, 512], BF16)
        nc.vector.memset(wu_sb, 0.0)
        zero_sb = cpool.tile([128, 1], F32)
        nc.vector.memset(zero_sb, 0.0)
        eps_sb = cpool.tile([128, 1], F32)
        nc.vector.memset(eps_sb, EPS)

        x_sb = xpool.tile([128, TB, KB, 128], BF16)
        w_sb = wpool.tile([128, KB, NOH, 512], BF16)

        # DMA schedule: the critical fill (x tile0 + all 8 per-ib w chunks)
        # is spread over all three DMA queues in consumption order so it
        # runs at the full HBM rate.  Bulk x for tiles 8-15 follows on the
        # SWDGE queue, gated behind w completion: the gpsimd copy below
        # READS every w chunk and WRITES into the bulk-x destination, so
        # the scheduler cannot hoist the bulk transfers over it (they'd
        # otherwise steal most of the HBM bandwidth from the fill).
        nc.sync.dma_start(out=x_sb[:, 0, 0:4, :], in_=xt_h[:, 0, 0:4, :])
        nc.scalar.dma_start(out=w_sb[:, 0:1], in_=wt_h[:, 0:1])
        nc.gpsimd.dma_start(out=w_sb[:, 2:3], in_=wt_h[:, 2:3])
        nc.sync.dma_start(out=w_sb[:, 1:2], in_=wt_h[:, 1:2])
        nc.scalar.dma_start(out=x_sb[:, 0, 4:8, :], in_=xt_h[:, 0, 4:8, :])
        nc.gpsimd.dma_start(out=w_sb[:, 5:6], in_=wt_h[:, 5:6])
        nc.sync.dma_start(out=w_sb[:, 4:5], in_=wt_h[:, 4:5])
        nc.scalar.dma_start(out=w_sb[:, 3:4], in_=wt_h[:, 3:4])
        nc.gpsimd.dma_start(out=w_sb[:, 7:8], in_=wt_h[:, 7:8])
        nc.scalar.dma_start(out=w_sb[:, 6:7], in_=wt_h[:, 6:7])
        nc.scalar.dma_start(out=x_sb[:, 1:2], in_=xt_h[:, 1:2])
        nc.sync.dma_start(out=x_sb[:, 2:4], in_=xt_h[:, 2:4])
        nc.sync.dma_start(out=x_sb[:, 4:8], in_=xt_h[:, 4:8])
        nc.gpsimd.tensor_copy(out=x_sb[:, 8:16, 0, 0:4],
                              in_=w_sb[:, 0:8, 0, 0:4])
        nc.gpsimd.dma_start(out=x_sb[:, 8:12], in_=xt_h[:, 8:12])
        nc.gpsimd.dma_start(out=x_sb[:, 12:16], in_=xt_h[:, 12:16])

        if with_nw:
            # norm_w broadcast to 128 partitions (general path only)
            nw_sb = cpool.tile([128, H], F32)
            nc.gpsimd.dma_start(
                out=nw_sb,
                in_=bass.AP(tensor=nw_h, offset=0, ap=[[0, 128], [1, H]]))

        # preload the ACT function table (Square/Sqrt) during the fill
        dummy = stats.tile([128, 1], F32)
        nc.scalar.activation(out=dummy, in_=zero_sb, func=AF.Square,
                             bias=zero_sb)

        # HAM warm-up: keep the PE busy from ~7us until the real matmuls
        # start so the clock gate opens to 8/8 before real work.
        wp = psum.tile([128, 512], F32, name="wp", tag="yp")
        for _ in range(N_WARM):
            nc.tensor.matmul(wp, wu_sb[:, 0:128], wu_sb,
                             start=True, stop=True)

        for tt in range(TB):
            # one psum BANK per output half -> finer-grained release
            yph = [psum.tile([128, 512], F32, tag="yp", name=f"yp{tt}_{oh}")
                   for oh in range(NOH)]
            for oh in range(NOH):
                for ib in range(KB):
                    nc.tensor.matmul(
                        yph[oh],
                        x_sb[:, tt, ib, :],
                        w_sb[:, ib, oh, :],
                        start=(ib == 0),
                        stop=(ib == KB - 1),
                    )
            # sum of squares over the hidden axis (free axis) on ACT,
            # one op per psum half so each waits on a single PE group
            sq = spool.tile([128, H], BF16, tag="sq")
            half_sums = stats.tile([128, 2], F32)
            for oh in range(NOH):
                sl = slice(oh * 512, (oh + 1) * 512)
                nc.scalar.activation(out=sq[:, sl], in_=yph[oh],
                                     func=AF.Square, bias=zero_sb,
                                     accum_out=half_sums[:, oh:oh + 1])
            ssum = stats.tile([128, 1], F32)
            nc.vector.tensor_add(out=ssum, in0=half_sums[:, 0:1],
                                 in1=half_sums[:, 1:2])
            # std = sqrt(mean + eps); rstd = 1/std
            std = stats.tile([128, 1], F32)
            nc.scalar.activation(out=std, in_=ssum, func=AF.Sqrt,
                                 bias=eps_sb, scale=1.0 / H)
            rstd = stats.tile([128, 1], F32)
            nc.vector.reciprocal(out=rstd, in_=std)
            # out = (y * rstd) [* norm_w], written bf16 on DVE; the two
            # halves leave via different DMA rings so the last tile drains
            # fast
            o_sb = opool.tile([128, H], BF16)
            if with_nw:
                for oh in range(NOH):
                    sl = slice(oh * 512, (oh + 1) * 512)
                    nc.vector.scalar_tensor_tensor(
                        out=o_sb[:, sl], in0=yph[oh], scalar=rstd,
                        in1=nw_sb[:, sl], op0=OP.mult, op1=OP.mult,
                    )
            else:
                for oh in range(NOH):
                    sl = slice(oh * 512, (oh + 1) * 512)
                    nc.vector.tensor_scalar_mul(out=o_sb[:, sl],
                                                in0=yph[oh],
                                                scalar1=rstd)
            rows = slice(tt * 128, (tt + 1) * 128)
            nc.sync.dma_start(out=out_h[rows, 0:512], in_=o_sb[:, 0:512])
            nc.scalar.dma_start(out=out_h[rows, 512:1024],
                                in_=o_sb[:, 512:1024])

    _legalize_multiwait(nc)
    return nc


def host_prep(x, conv_w, norm_w, with_nw):
    """Shard + lay out the full inputs into per-core device input maps."""
    bf16 = ml_dtypes.bfloat16

    # Collapse the 20 1x1 convs: W[o,i] = sum_l conv_w[l,o,i] / L
    w = np.asarray(conv_w).sum(axis=0) * (1.0 / L)          # [H(o), H(i)] f32
    # wt[p, ib, oh, j] = W[oh*512+j, ib*128+p]
    wt = np.ascontiguousarray(
        w.reshape(NOH, 512, KB, 128).transpose(3, 2, 0, 1).astype(bf16))

    x2d = np.asarray(x).reshape(TOK, H)
    xbf = x2d.astype(bf16)

    in_maps = []
    for c in range(N_CORES):
        xc = xbf[c * TPC:(c + 1) * TPC]                      # [TPC, H]
        # xt[p, tt, ib, t] = xc[tt*128+t, ib*128+p]
        xtc = np.ascontiguousarray(
            xc.reshape(TB, 128, KB, 128).transpose(3, 0, 2, 1))
        m = {"xt": xtc, "wt": wt}
        if with_nw:
            m["nw"] = np.ascontiguousarray(np.asarray(norm_w),
                                           dtype=np.float32)
        in_maps.append(m)
    return in_maps


def kernel(x, conv_w, norm_w):
    global LAST_RESULTS

    x = np.asarray(x)
    out_dtype = x.dtype
    nw = np.asarray(norm_w)
    with_nw = not bool(np.all(nw == 1.0))

    if with_nw not in _BUILT:
        _BUILT[with_nw] = _build(with_nw)
    nc = _BUILT[with_nw]

    in_maps = host_prep(x, conv_w, norm_w, with_nw)

    res = run_bass_kernel_spmd(nc, in_maps, core_ids=list(range(N_CORES)))
    LAST_RESULTS = res

    out = np.concatenate([r["out"] for r in res.results], axis=0)
    return out.reshape(B, S, H).astype(out_dtype, copy=False)


# revision 23
# speedup vs baseline: 1.2280x; 1.0744x over previous
"""Trainium2 Bass kernel for Conv2D(sum of 20 1x1 convs) + QwenRMSNorm.

Math: y = einsum("bsi,loi->bso", x, conv_w) / L ; out = rmsnorm(y) * norm_w.
Since x does not depend on l, the 20-matrix contraction collapses to a single
matmul with W = sum_l conv_w[l] / L.  Host pre-sums/transposes/casts the weight
(one [H,H] matrix) and lays out x as token-sharded, hidden-major bf16 slabs;
the 8 NeuronCores each run matmul (bf16, fp32 accum) + RMSNorm on their 2048
tokens.  All device compute is token-local; no collectives.

v2 layout/schedule notes:
 - DRAM layouts are partition-outermost so multi-tile DMA chunks are
   contiguous per partition (x in 4 chunks, w in 4 chunks split across both
   HWDGE rings so the critical w+x0 fill gets full HBM bandwidth).
 - ~10 warm-up matmuls on a memset tile run during the DMA fill so the PE
   HAM clock-gate is at 8/8 before the first real matmul.
 - A dummy activation preloads the ACT function table during the fill.
 - Output is written bf16 (host upcasts); rel-err cost ~1e-3.
 - norm_w == 1 (the spec's fill) skips the [128,H] norm_w broadcast and the
   per-tile tensor_tensor multiply; a general variant handles arbitrary
   norm_w.
"""

import numpy as np
import ml_dtypes
from contextlib import ExitStack

import concourse.bass as bass
import concourse.mybir as mybir
import concourse.tile as tile
from concourse.bass_utils import run_bass_kernel_spmd

N_CORES = 8
B, S, H, L = 4, 4096, 1024, 20
TOK = B * S               # 16384 tokens
TPC = TOK // N_CORES      # 2048 tokens per core
TB = TPC // 128           # 16 token-blocks of 128 per core
KB = H // 128             # 8 contraction blocks
NOH = H // 512            # 2 psum halves of the output row
EPS = 1e-6
N_WARM = 8                # HAM warm-up matmuls (N=512) during the DMA fill

BF16 = mybir.dt.bfloat16
F32 = mybir.dt.float32
AF = mybir.ActivationFunctionType
OP = mybir.AluOpType

_BUILT = {}          # variant -> cached Bass program
LAST_RESULTS = None  # BassKernelResults of the most recent run (for test harness)


def _legalize_multiwait(nc):
    """The walrus build here encodes exactly one semaphore wait per 64B
    instruction (NEURON_ISA_TPB_EVENTS has a single wait slot) and errors on
    Tile's multi-wait instructions.  Split surplus waits into standalone
    EVENT_SEMAPHORE instructions on the same engine, placed directly before
    the original instruction (same sequencer stream -> same semantics)."""
    n_ev = 0
    for f in nc.m.functions:
        for blk in f.blocks:
            insts = blk.instructions
            out = []
            changed = False
            for inst in list(insts):
                si = getattr(inst, "sync_info", None)
                waits = list(si.on_wait) if si is not None else []
                if len(waits) > 1:
                    changed = True
                    updates = list(si.on_update)
                    for w in waits[:-1]:
                        ev = mybir.InstEventSemaphore(
                            name=f"{inst.name}-sw{n_ev}", ins=[], outs=[])
                        n_ev += 1
                        ev.engine = inst.engine
                        ev.sync_info = mybir.SyncInfo(on_wait=[w], on_update=[])
                        out.append(ev)
                    inst.sync_info = mybir.SyncInfo(
                        on_wait=[waits[-1]], on_update=updates)
                out.append(inst)
            if changed:
                insts.clear()
                insts.extend(out)


def _build(with_nw):
    nc = bass.Bass()
    # x layout (partition-outermost): xt[p, tt, ib, t] = x[tt*128+t, ib*128+p]
    xt_h = nc.dram_tensor("xt", [128, TB, KB, 128], BF16, kind="ExternalInput")
    # w layout: wt[p, ib, oh, j] = W[oh*512+j, ib*128+p], W = sum_l conv_w[l]/L
    # (ib-major so one 512KB chunk = an ib-pair for BOTH output halves, with
    # 4KB contiguous per partition)
    wt_h = nc.dram_tensor("wt", [128, KB, NOH, 512], BF16, kind="ExternalInput")
    if with_nw:
        nw_h = nc.dram_tensor("nw", [H], F32, kind="ExternalInput")
    out_h = nc.dram_tensor("out", [TPC, H], BF16, kind="ExternalOutput")

    with tile.TileContext(nc) as tc, ExitStack() as ctx:
        xpool = ctx.enter_context(tc.tile_pool(name="x", bufs=1))
        wpool = ctx.enter_context(tc.tile_pool(name="w", bufs=1))
        cpool = ctx.enter_context(tc.tile_pool(name="consts", bufs=1))
        opool = ctx.enter_context(tc.tile_pool(name="out", bufs=6))
        spool = ctx.enter_context(tc.tile_pool(name="scratch", bufs=2))
        stats = ctx.enter_context(tc.tile_pool(name="stats", bufs=8))
        psum = ctx.enter_context(tc.tile_pool(name="psum", bufs=8, space="PSUM"))

        # const tiles first so the warm-up matmuls' source is ready early
        wu_sb = cpool.tile([128, 512], BF16)
        nc.vector.memset(wu_sb, 0.0)
        zero_sb = cpool.tile([128, 1], F32)
        nc.vector.memset(zero_sb, 0.0)
        eps_sb = cpool.tile([128, 1], F32)
        nc.vector.memset(eps_sb, EPS)

        x_sb = xpool.tile([128, TB, KB, 128], BF16)
        w_sb = wpool.tile([128, KB, NOH, 512], BF16)

        # DMA schedule: per-queue bandwidth scales with per-partition
        # contiguity (2KB-chunk transfers crawl at ~150GB/s, 8KB+ run at
        # ~400GB/s), so use few LARGE transfers: w as two 1MB ib-halves on
        # the scalar ring (the matmul loop consumes w in matching halves),
        # x in three large chunks on the sync ring, bulk x8-15 behind w on
        # the scalar ring.
        nc.sync.dma_start(out=x_sb[:, 0:2], in_=xt_h[:, 0:2])
        nc.scalar.dma_start(out=w_sb[:, 0:4], in_=wt_h[:, 0:4])
        nc.scalar.dma_start(out=w_sb[:, 4:8], in_=wt_h[:, 4:8])
        nc.sync.dma_start(out=x_sb[:, 2:8], in_=xt_h[:, 2:8])
        nc.scalar.dma_start(out=x_sb[:, 8:16], in_=xt_h[:, 8:16])

        if with_nw:
            # norm_w broadcast to 128 partitions (general path only)
            nw_sb = cpool.tile([128, H], F32)
            nc.gpsimd.dma_start(
                out=nw_sb,
                in_=bass.AP(tensor=nw_h, offset=0, ap=[[0, 128], [1, H]]))

        # preload the ACT function table (Square/Sqrt) during the fill
        dummy = stats.tile([128, 1], F32)
        nc.scalar.activation(out=dummy, in_=zero_sb, func=AF.Square,
                             bias=zero_sb)

        # HAM warm-up: keep the PE busy from ~7us until the real matmuls
        # start so the clock gate opens to 8/8 before real work.
        wp = psum.tile([128, 512], F32, name="wp", tag="yp")
        for _ in range(N_WARM):
            nc.tensor.matmul(wp, wu_sb[:, 0:128], wu_sb,
                             start=True, stop=True)

        for tt in range(TB):
            # one psum BANK per output half -> finer-grained release; the
            # ib loop is split in halves matching the two w DMA chunks so
            # the first 8 matmuls start as soon as w[0:4] lands
            yph = [psum.tile([128, 512], F32, tag="yp", name=f"yp{tt}_{oh}")
                   for oh in range(NOH)]
            for ibh in range(2):
                for oh in range(NOH):
                    for ib in range(ibh * 4, ibh * 4 + 4):
                        nc.tensor.matmul(
                            yph[oh],
                            x_sb[:, tt, ib, :],
                            w_sb[:, ib, oh, :],
                            start=(ib == 0),
                            stop=(ib == KB - 1),
                        )
            # sum of squares over the hidden axis (free axis) on ACT,
            # one op per psum half so each waits on a single PE group
            sq = spool.tile([128, H], BF16, tag="sq")
            half_sums = stats.tile([128, 2], F32)
            for oh in range(NOH):
                sl = slice(oh * 512, (oh + 1) * 512)
                nc.scalar.activation(out=sq[:, sl], in_=yph[oh],
                                     func=AF.Square, bias=zero_sb,
                                     accum_out=half_sums[:, oh:oh + 1])
            ssum = stats.tile([128, 1], F32)
            nc.vector.tensor_add(out=ssum, in0=half_sums[:, 0:1],
                                 in1=half_sums[:, 1:2])
            # std = sqrt(mean + eps); rstd = 1/std
            std = stats.tile([128, 1], F32)
            nc.scalar.activation(out=std, in_=ssum, func=AF.Sqrt,
                                 bias=eps_sb, scale=1.0 / H)
            rstd = stats.tile([128, 1], F32)
            nc.vector.reciprocal(out=rstd, in_=std)
            # out = (y * rstd) [* norm_w], written bf16 on DVE; the two
            # halves leave via different DMA rings so the last tile drains
            # fast
            o_sb = opool.tile([128, H], BF16)
            if with_nw:
                for oh in range(NOH):
                    sl = slice(oh * 512, (oh + 1) * 512)
                    nc.vector.scalar_tensor_tensor(
                        out=o_sb[:, sl], in0=yph[oh], scalar=rstd,
                        in1=nw_sb[:, sl], op0=OP.mult, op1=OP.mult,
                    )
            else:
                for oh in range(NOH):
                    sl = slice(oh * 512, (oh + 1) * 512)
                    nc.vector.tensor_scalar_mul(out=o_sb[:, sl],
                                                in0=yph[oh],
                                                scalar1=rstd)
            rows = slice(tt * 128, (tt + 1) * 128)
            nc.sync.dma_start(out=out_h[rows, 0:512], in_=o_sb[:, 0:512])
            nc.scalar.dma_start(out=out_h[rows, 512:1024],
                                in_=o_sb[:, 512:1024])

    _legalize_multiwait(nc)
    return nc


def host_prep(x, conv_w, norm_w, with_nw):
    """Shard + lay out the full inputs into per-core device input maps."""
    bf16 = ml_dtypes.bfloat16

    # Collapse the 20 1x1 convs: W[o,i] = sum_l conv_w[l,o,i] / L
    w = np.asarray(conv_w).sum(axis=0) * (1.0 / L)          # [H(o), H(i)] f32
    # wt[p, ib, oh, j] = W[oh*512+j, ib*128+p]
    wt = np.ascontiguousarray(
        w.reshape(NOH, 512, KB, 128).transpose(3, 2, 0, 1).astype(bf16))

    x2d = np.asarray(x).reshape(TOK, H)
    xbf = x2d.astype(bf16)

    in_maps = []
    for c in range(N_CORES):
        xc = xbf[c * TPC:(c + 1) * TPC]                      # [TPC, H]
        # xt[p, tt, ib, t] = xc[tt*128+t, ib*128+p]
        xtc = np.ascontiguousarray(
            xc.reshape(TB, 128, KB, 128).transpose(3, 0, 2, 1))
        m = {"xt": xtc, "wt": wt}
        if with_nw:
            m["nw"] = np.ascontiguousarray(np.asarray(norm_w),
                                           dtype=np.float32)
        in_maps.append(m)
    return in_maps


def kernel(x, conv_w, norm_w):
    global LAST_RESULTS

    x = np.asarray(x)
    out_dtype = x.dtype
    nw = np.asarray(norm_w)
    with_nw = not bool(np.all(nw == 1.0))

    if with_nw not in _BUILT:
        _BUILT[with_nw] = _build(with_nw)
    nc = _BUILT[with_nw]

    in_maps = host_prep(x, conv_w, norm_w, with_nw)

    res = run_bass_kernel_spmd(nc, in_maps, core_ids=list(range(N_CORES)))
    LAST_RESULTS = res

    out = np.concatenate([r["out"] for r in res.results], axis=0)
    return out.reshape(B, S, H).astype(out_dtype, copy=False)


# revision 26
# speedup vs baseline: 1.3114x; 1.0679x over previous
"""Trainium2 Bass kernel for Conv2D(sum of 20 1x1 convs) + QwenRMSNorm.

Math: y = einsum("bsi,loi->bso", x, conv_w) / L ; out = rmsnorm(y) * norm_w.
Since x does not depend on l, the 20-matrix contraction collapses to a single
matmul with W = sum_l conv_w[l] / L.  Host pre-sums/transposes/casts the weight
(one [H,H] matrix) and lays out x as token-sharded, hidden-major bf16 slabs;
the 8 NeuronCores each run matmul (bf16, fp32 accum) + RMSNorm on their 2048
tokens.  All device compute is token-local; no collectives.

v2 layout/schedule notes:
 - DRAM layouts are partition-outermost so multi-tile DMA chunks are
   contiguous per partition (x in 4 chunks, w in 4 chunks split across both
   HWDGE rings so the critical w+x0 fill gets full HBM bandwidth).
 - ~10 warm-up matmuls on a memset tile run during the DMA fill so the PE
   HAM clock-gate is at 8/8 before the first real matmul.
 - A dummy activation preloads the ACT function table during the fill.
 - Output is written bf16 (host upcasts); rel-err cost ~1e-3.
 - norm_w == 1 (the spec's fill) skips the [128,H] norm_w broadcast and the
   per-tile tensor_tensor multiply; a general variant handles arbitrary
   norm_w.
"""

import numpy as np
import ml_dtypes
from contextlib import ExitStack

import concourse.bass as bass
import concourse.mybir as mybir
import concourse.tile as tile
from concourse.bass_utils import run_bass_kernel_spmd

N_CORES = 8
B, S, H, L = 4, 4096, 1024, 20
TOK = B * S               # 16384 tokens
TPC = TOK // N_CORES      # 2048 tokens per core
TB = TPC // 128           # 16 token-blocks of 128 per core
KB = H // 128             # 8 contraction blocks
NOH = H // 512            # 2 psum halves of the output row
EPS = 1e-6
N_WARM = 8                # HAM warm-up matmuls (N=512) during the DMA fill

BF16 = mybir.dt.bfloat16
F32 = mybir.dt.float32
AF = mybir.ActivationFunctionType
OP = mybir.AluOpType

_BUILT = {}          # variant -> cached Bass program
LAST_RESULTS = None  # BassKernelResults of the most recent run (for test harness)


def _legalize_multiwait(nc):
    """The walrus build here encodes exactly one semaphore wait per 64B
    instruction (NEURON_ISA_TPB_EVENTS has a single wait slot) and errors on
    Tile's multi-wait instructions.  Split surplus waits into standalone
    EVENT_SEMAPHORE instructions on the same engine, placed directly before
    the original instruction (same sequencer stream -> same semantics)."""
    n_ev = 0
    for f in nc.m.functions:
        for blk in f.blocks:
            insts = blk.instructions
            out = []
            changed = False
            for inst in list(insts):
                si = getattr(inst, "sync_info", None)
                waits = list(si.on_wait) if si is not None else []
                if len(waits) > 1:
                    changed = True
                    updates = list(si.on_update)
                    for w in waits[:-1]:
                        ev = mybir.InstEventSemaphore(
                            name=f"{inst.name}-sw{n_ev}", ins=[], outs=[])
                        n_ev += 1
                        ev.engine = inst.engine
                        ev.sync_info = mybir.SyncInfo(on_wait=[w], on_update=[])
                        out.append(ev)
                    inst.sync_info = mybir.SyncInfo(
                        on_wait=[waits[-1]], on_update=updates)
                out.append(inst)
            if changed:
                insts.clear()
                insts.extend(out)


def _build(with_nw):
    nc = bass.Bass()
    # x layout (partition-outermost): xt[p, tt, ib, t] = x[tt*128+t, ib*128+p]
    xt_h = nc.dram_tensor("xt", [128, TB, KB, 128], BF16, kind="ExternalInput")
    # w layout: wt[p, ib, oh, j] = W[oh*512+j, ib*128+p], W = sum_l conv_w[l]/L
    # (ib-major so one 512KB chunk = an ib-pair for BOTH output halves, with
    # 4KB contiguous per partition)
    wt_h = nc.dram_tensor("wt", [128, KB, NOH, 512], BF16, kind="ExternalInput")
    if with_nw:
        nw_h = nc.dram_tensor("nw", [H], F32, kind="ExternalInput")
    out_h = nc.dram_tensor("out", [TPC, H], BF16, kind="ExternalOutput")

    with tile.TileContext(nc) as tc, ExitStack() as ctx:
        xpool = ctx.enter_context(tc.tile_pool(name="x", bufs=1))
        wpool = ctx.enter_context(tc.tile_pool(name="w", bufs=1))
        cpool = ctx.enter_context(tc.tile_pool(name="consts", bufs=1))
        opool = ctx.enter_context(tc.tile_pool(name="out", bufs=6))
        spool = ctx.enter_context(tc.tile_pool(name="scratch", bufs=2))
        stats = ctx.enter_context(tc.tile_pool(name="stats", bufs=8))
        psum = ctx.enter_context(tc.tile_pool(name="psum", bufs=4, space="PSUM"))

        # const tiles first so the warm-up matmuls' source is ready early
        wu_sb = cpool.tile([128, 512], BF16)
        nc.vector.memset(wu_sb, 0.0)
        zero_sb = cpool.tile([128, 1], F32)
        nc.vector.memset(zero_sb, 0.0)
        eps_sb = cpool.tile([128, 1], F32)
        nc.vector.memset(eps_sb, EPS)

        x_sb = xpool.tile([128, TB, KB, 128], BF16)
        w_sb = wpool.tile([128, KB, NOH, 512], BF16)

        # DMA schedule: per-queue bandwidth scales with per-partition
        # contiguity (2KB-chunk transfers crawl at ~150GB/s, 8KB+ run at
        # ~400GB/s), so use few LARGE transfers, and keep ALL bulk x
        # behind w on the same ring so nothing competes with the critical
        # fill (x tile0/1 + w).  The sync ring then only carries out-half0
        # and the gpsimd queue out-half1.
        nc.sync.dma_start(out=x_sb[:, 0:2], in_=xt_h[:, 0:2])
        nc.scalar.dma_start(out=w_sb[:, 0:4], in_=wt_h[:, 0:4])
        nc.scalar.dma_start(out=w_sb[:, 4:8], in_=wt_h[:, 4:8])
        nc.scalar.dma_start(out=x_sb[:, 2:8], in_=xt_h[:, 2:8])
        nc.scalar.dma_start(out=x_sb[:, 8:16], in_=xt_h[:, 8:16])

        if with_nw:
            # norm_w broadcast to 128 partitions (general path only)
            nw_sb = cpool.tile([128, H], F32)
            nc.gpsimd.dma_start(
                out=nw_sb,
                in_=bass.AP(tensor=nw_h, offset=0, ap=[[0, 128], [1, H]]))

        # preload the ACT function table (Square/Sqrt) during the fill
        dummy = stats.tile([128, 1], F32)
        nc.scalar.activation(out=dummy, in_=zero_sb, func=AF.Square,
                             bias=zero_sb)

        # HAM warm-up: keep the PE busy from ~7us until the real matmuls
        # start so the clock gate opens to 8/8 before real work.
        wp = psum.tile([128, H], F32, name="wp", tag="yp")
        for _ in range(N_WARM):
            nc.tensor.matmul(wp[:, 0:512], wu_sb[:, 0:128], wu_sb,
                             start=True, stop=True)

        for tt in range(TB):
            # the ib loop is split in halves matching the two w DMA chunks
            # so the first 8 matmuls start as soon as w[0:4] lands
            yp = psum.tile([128, H], F32, tag="yp")
            for ibh in range(2):
                for oh in range(NOH):
                    for ib in range(ibh * 4, ibh * 4 + 4):
                        nc.tensor.matmul(
                            yp[:, oh * 512:(oh + 1) * 512],
                            x_sb[:, tt, ib, :],
                            w_sb[:, ib, oh, :],
                            start=(ib == 0),
                            stop=(ib == KB - 1),
                        )
            # sum of squares over the whole hidden row in ONE ACT op: the
            # norm chain is latency-bound by cross-engine hops, so keep it
            # ACT(square+acc, sqrt) -> DVE(recip, scale) with no ping-pong
            sq = spool.tile([128, H], BF16, tag="sq")
            ssum = stats.tile([128, 1], F32)
            nc.scalar.activation(out=sq, in_=yp, func=AF.Square,
                                 bias=zero_sb, accum_out=ssum)
            # std = sqrt(mean + eps); rstd = 1/std
            std = stats.tile([128, 1], F32)
            nc.scalar.activation(out=std, in_=ssum, func=AF.Sqrt,
                                 bias=eps_sb, scale=1.0 / H)
            rstd = stats.tile([128, 1], F32)
            nc.vector.reciprocal(out=rstd, in_=std)
            # out = (y * rstd) [* norm_w], written bf16 on DVE; the two
            # halves leave via different DMA queues (sync / gpsimd -- both
            # otherwise idle) so the last tile drains fast
            o_sb = opool.tile([128, H], BF16)
            if with_nw:
                for oh in range(NOH):
                    sl = slice(oh * 512, (oh + 1) * 512)
                    nc.vector.scalar_tensor_tensor(
                        out=o_sb[:, sl], in0=yp[:, sl], scalar=rstd,
                        in1=nw_sb[:, sl], op0=OP.mult, op1=OP.mult,
                    )
            else:
                for oh in range(NOH):
                    sl = slice(oh * 512, (oh + 1) * 512)
                    nc.vector.tensor_scalar_mul(out=o_sb[:, sl],
                                                in0=yp[:, sl],
                                                scalar1=rstd)
            rows = slice(tt * 128, (tt + 1) * 128)
            nc.sync.dma_start(out=out_h[rows, 0:512], in_=o_sb[:, 0:512])
            nc.gpsimd.dma_start(out=out_h[rows, 512:1024],
                                in_=o_sb[:, 512:1024])

    _legalize_multiwait(nc)
    return nc


def host_prep(x, conv_w, norm_w, with_nw):
    """Shard + lay out the full inputs into per-core device input maps."""
    bf16 = ml_dtypes.bfloat16

    # Collapse the 20 1x1 convs: W[o,i] = sum_l conv_w[l,o,i] / L
    w = np.asarray(conv_w).sum(axis=0) * (1.0 / L)          # [H(o), H(i)] f32
    # wt[p, ib, oh, j] = W[oh*512+j, ib*128+p]
    wt = np.ascontiguousarray(
        w.reshape(NOH, 512, KB, 128).transpose(3, 2, 0, 1).astype(bf16))

    x2d = np.asarray(x).reshape(TOK, H)
    xbf = x2d.astype(bf16)

    in_maps = []
    for c in range(N_CORES):
        xc = xbf[c * TPC:(c + 1) * TPC]                      # [TPC, H]
        # xt[p, tt, ib, t] = xc[tt*128+t, ib*128+p]
        xtc = np.ascontiguousarray(
            xc.reshape(TB, 128, KB, 128).transpose(3, 0, 2, 1))
        m = {"xt": xtc, "wt": wt}
        if with_nw:
            m["nw"] = np.ascontiguousarray(np.asarray(norm_w),
                                           dtype=np.float32)
        in_maps.append(m)
    return in_maps


def kernel(x, conv_w, norm_w):
    global LAST_RESULTS

    x = np.asarray(x)
    out_dtype = x.dtype
    nw = np.asarray(norm_w)
    with_nw = not bool(np.all(nw == 1.0))

    if with_nw not in _BUILT:
        _BUILT[with_nw] = _build(with_nw)
    nc = _BUILT[with_nw]

    in_maps = host_prep(x, conv_w, norm_w, with_nw)

    res = run_bass_kernel_spmd(nc, in_maps, core_ids=list(range(N_CORES)))
    LAST_RESULTS = res

    out = np.concatenate([r["out"] for r in res.results], axis=0)
    return out.reshape(B, S, H).astype(out_dtype, copy=False)
